# revision 1
# baseline (speedup 1.0000x reference)
"""Trainium2 Bass kernel for nn_MultiModalFusionModelWithAblation.

Strategy: pure data-parallel over 8 NeuronCores (B=16384 -> 2048 rows/core).
Row-major home layout ([rows<=128 partitions, features free]); all matmuls
take the activation as the stationary operand (lhsT, feature-major via bf16
DMA-transpose) and the weight as the moving operand, so outputs land
row-major in PSUM.  bf16 matmul inputs, fp32 PSUM accumulation.

Host-side algebra (exact, weight-space only):
  - gat_W folded into the MHA score/value projections: Wh/ctx/k are never
    materialized.  scores use GS = gat_W @ [A_emo|A_pkl|a1|a2] (A from the
    per-head query vectors, incl. 1/sqrt(HD)); values use GV = gat_W @ Wv.
  - LN affines folded into downstream weights where linear.
  - all per-output-feature biases handled by K=1 ones-outer-product matmuls
    into PSUM (skipped entirely when the bias is zero, the spec default).
"""
import sys
import os

sys.path.insert(0, "/opt/trn_rl_repo")

import numpy as np
import orjson
from contextlib import ExitStack

import concourse.bass as bass
import concourse.tile as tile
from concourse import mybir
from concourse.masks import make_identity

# ----------------------------------------------------------------------------
# walrus on this toolchain rejects >1 sync-wait per instruction; split excess
# waits onto NoOp carriers on the same engine queue (in-order => equivalent).
_FIXN = [0]


def _fix_bir_waits(d):
    for f in d.get("functions", []):
        for b in f.get("blocks", []):
            insts = b.get("instructions", [])
            if not any(
                len(((i.get("sync_info") or {}).get("on_wait") or [])) > 1
                for i in insts
            ):
                continue
            new = []
            for inst in insts:
                si = inst.get("sync_info")
                waits = (si or {}).get("on_wait") or []
                if len(waits) > 1:
                    for w in waits[:-1]:
                        _FIXN[0] += 1
                        new.append({
                            "engine": inst["engine"], "ins": [], "outs": [],
                            "name": f"wfix-{_FIXN[0]}", "opcode": "NoOp",
                            "debug": inst.get("debug", 0),
                            "sync_info": {"on_update": [], "on_wait": [w]},
                        })
                    si["on_wait"] = [waits[-1]]
                new.append(inst)
            b["instructions"] = new
    return d


if not getattr(bass.Bass, "_waitfix_installed", False):
    _orig_tjb = bass.Bass.to_json_bytes

    def _patched_tjb(self):
        return orjson.dumps(_fix_bir_waits(orjson.loads(_orig_tjb(self))))

    bass.Bass.to_json_bytes = _patched_tjb
    bass.Bass._waitfix_installed = True

# ----------------------------------------------------------------------------
H = 512
NH = 8
HD = 64
NMOD = 5
IN_DIMS = [2048, 1024, 1536, 512, 512]
MODS = ["body", "face", "scene", "audio", "text"]
B_FULL = 16384
NCORES = 8
B_CORE = B_FULL // NCORES          # 2048
NT = B_CORE // 128                 # 16 row tiles per core
ALPHA = 0.2
EPS = 1e-5

F32 = mybir.dt.float32
BF16 = mybir.dt.bfloat16
AF = mybir.ActivationFunctionType
AL = mybir.AluOpType


def _build_nc(flags, repeat=1):
    """Build the SPMD per-core Bass program. `flags` has booleans for the
    optional bias paths (all False for the spec's zero-filled biases).
    `repeat` re-runs the whole row loop (timing experiments only)."""
    nc = bass.Bass("TRN2", target_bir_lowering=False, debug=False,
                   num_devices=NCORES)

    # ---- dram io ----
    feat_d = [nc.dram_tensor(f"feat_{m}", [B_CORE, ind], F32, kind="ExternalInput")
              for m, ind in zip(MODS, IN_DIMS)]
    wp_d = [nc.dram_tensor(f"wp_{m}", [ind, H], F32, kind="ExternalInput")
            for m, ind in zip(MODS, IN_DIMS)]
    aw1_d = nc.dram_tensor("aw1", [NMOD, H, H // 2], F32, kind="ExternalInput")
    aw2_d = nc.dram_tensor("aw2", [NMOD, H // 2, H], F32, kind="ExternalInput")
    gv_d = nc.dram_tensor("gv", [H, H], F32, kind="ExternalInput")
    gs_d = nc.dram_tensor("gs", [H, 18], F32, kind="ExternalInput")
    wo_d = nc.dram_tensor("wo", [H, H], F32, kind="ExternalInput")
    pc_d = nc.dram_tensor("pc", [H, 24], F32, kind="ExternalInput")
    elp5_d = nc.dram_tensor("elp5", [35, H], F32, kind="ExternalInput")
    plp5_d = nc.dram_tensor("plp5", [25, H], F32, kind="ExternalInput")
    logits_d = nc.dram_tensor("logits", [NMOD, B_CORE, 7], F32, kind="ExternalInput")
    scores5_d = nc.dram_tensor("scores5", [NMOD, B_CORE, 5], F32, kind="ExternalInput")
    # optional bias rows (always declared; tiny)
    bp_d = nc.dram_tensor("bp", [NMOD, H], F32, kind="ExternalInput")
    ab1_d = nc.dram_tensor("ab1e", [NMOD, H // 2], F32, kind="ExternalInput")
    ab2_d = nc.dram_tensor("ab2e", [NMOD, H], F32, kind="ExternalInput")
    rc_d = nc.dram_tensor("rc", [2, H], F32, kind="ExternalInput")
    pcb_d = nc.dram_tensor("pcb", [1, 24], F32, kind="ExternalInput")
    ck_d = nc.dram_tensor("ck", [1, 16], F32, kind="ExternalInput")
    out_d = nc.dram_tensor("out", [B_CORE, 12], F32, kind="ExternalOutput")

    NK = [ind // 128 for ind in IN_DIMS]

    with tile.TileContext(nc) as tc, ExitStack() as ctx:
        wp_pool = ctx.enter_context(tc.tile_pool(name="weights", bufs=1))
        sb = ctx.enter_context(tc.tile_pool(name="work", bufs=1))
        ps = ctx.enter_context(tc.tile_pool(name="psum", bufs=1, space="PSUM"))

        # ---- one-time weight loads: fast HWDGE fp32 loads, cast on DVE/ACT
        # (SWDGE cast-DMA serializes ~35us of Q7 descriptor generation).
        _cast_i = [0]

        def _load_cast_bf16(dram_ap_3d, shape_c, tag):
            # dram_ap_3d: [128, K, N] fp32 view; returns bf16 tile same shape
            t = wp_pool.tile(shape_c, BF16, tag=tag)
            kdim = shape_c[1]
            for k0 in range(0, kdim, 2):
                k1 = min(k0 + 2, kdim)
                stg = sb.tile([shape_c[0], 2, shape_c[2]], F32, tag="wstage",
                              bufs=2)
                nc.sync.dma_start(stg[:, :k1 - k0, :], dram_ap_3d[:, k0:k1, :])
                if _cast_i[0] % 2:
                    nc.vector.tensor_copy(out=t[:, k0:k1, :],
                                          in_=stg[:, :k1 - k0, :])
                else:
                    nc.scalar.copy(out=t[:, k0:k1, :], in_=stg[:, :k1 - k0, :])
                _cast_i[0] += 1
            return t

        wp_bf = [
            _load_cast_bf16(
                wp_d[m].ap().rearrange("(k c) n -> c k n", c=128),
                [128, NK[m], H], f"wp{m}")
            for m in range(NMOD)
        ]
        aw1_bf = _load_cast_bf16(
            aw1_d.ap().rearrange("m (k c) n -> c (m k) n", c=128),
            [128, NMOD * 4, H // 2], "aw1")
        aw1_bf = aw1_bf[:].rearrange("c (m k) n -> c m k n", m=NMOD)
        aw2_bf = _load_cast_bf16(
            aw2_d.ap().rearrange("m (k c) n -> c (m k) n", c=128),
            [128, NMOD * 2, H], "aw2")
        aw2_bf = aw2_bf[:].rearrange("c (m k) n -> c m k n", m=NMOD)
        gv_bf = _load_cast_bf16(
            gv_d.ap().rearrange("(k c) n -> c k n", c=128), [128, 4, H], "gv")
        gs_bf = _load_cast_bf16(
            gs_d.ap().rearrange("(k c) n -> c k n", c=128), [128, 4, 18], "gs")
        wo_bf = _load_cast_bf16(
            wo_d.ap().rearrange("(k c) n -> c k n", c=128), [128, 4, H], "wo")
        pc_bf = _load_cast_bf16(
            pc_d.ap().rearrange("(k c) n -> c k n", c=128), [128, 4, 24], "pc")
        elp5_bf = _load_cast_bf16(elp5_d.ap()[:, None, :], [35, 1, H], "elp5")
        elp5_bf = elp5_bf[:, 0, :]
        plp5_bf = _load_cast_bf16(plp5_d.ap()[:, None, :], [25, 1, H], "plp5")
        plp5_bf = plp5_bf[:, 0, :]

        ident = wp_pool.tile([128, 128], BF16, tag="ident")
        make_identity(nc, ident[:])
        eps_t = wp_pool.tile([128, 1], F32, tag="eps")
        nc.vector.memset(eps_t[:], EPS)

        ones1 = None
        if any([flags["bp"], flags["ab1"], flags["ab2"], flags["rc"],
                flags["pcb"], flags["ck"]]):
            ones1 = wp_pool.tile([1, 128], BF16, tag="ones1")
            nc.vector.memset(ones1[:], 1.0)

        def _bias_row(dram_ap, n, tag):
            t = wp_pool.tile([1, n], BF16, tag=tag)
            nc.gpsimd.dma_start(t[:], dram_ap)
            return t

        bp_bf = _bias_row(bp_d.ap().rearrange("m n -> 1 (m n)"), NMOD * H, "bp") \
            if flags["bp"] else None
        ab1_bf = _bias_row(ab1_d.ap().rearrange("m n -> 1 (m n)"), NMOD * 256, "ab1") \
            if flags["ab1"] else None
        ab2_bf = _bias_row(ab2_d.ap().rearrange("m n -> 1 (m n)"), NMOD * H, "ab2") \
            if flags["ab2"] else None
        rc_bf = _bias_row(rc_d.ap().rearrange("q n -> 1 (q n)"), 2 * H, "rc") \
            if flags["rc"] else None
        pcb_bf = _bias_row(pcb_d.ap()[:], 24, "pcb") if flags["pcb"] else None
        ck_t = None
        if flags["ck"]:
            ck_row = _bias_row(ck_d.ap()[:], 16, "ckrow")
            ck_ps = ps.tile([128, 16], F32, tag="psB")
            nc.tensor.matmul(ck_ps[:], lhsT=ones1[:], rhs=ck_row[:],
                             start=True, stop=True)
            ck_t = wp_pool.tile([128, 16], F32, tag="ckt")
            nc.vector.tensor_copy(out=ck_t[:], in_=ck_ps[:])

        # ---------------- per row-tile pipeline ----------------
        # Software pipelined: emit order is A(rt), TAIL(rt-1), B..E(rt) so the
        # PE queue never head-of-line blocks on the previous tile's
        # vector-heavy attention tail, and vice versa.

        # persistent zero-padded staging for the aux-logit transposes
        lgpad = []
        scpad = []
        for i in range(2):
            t = wp_pool.tile([128, 128], BF16, tag=f"lgpad{i}")
            nc.vector.memset(t[:], 0.0)
            lgpad.append(t)
            t2 = wp_pool.tile([128, 128], BF16, tag=f"scpad{i}")
            nc.vector.memset(t2[:], 0.0)
            scpad.append(t2)

        def layer_norm(src_tiles, dst_tag, t6, t2, tsd, trs):
            # outputs land in one contiguous [128, NMOD, H] tile so the
            # downstream feature-major transpose is a single xbar call
            out = sb.tile([128, NMOD, H], BF16, tag=dst_tag, bufs=2)
            stats = []
            for m in range(NMOD):
                st6 = sb.tile([128, 6], F32, tag=t6, bufs=6)
                nc.vector.bn_stats(st6[:], src_tiles[m][:])
                st2 = sb.tile([128, 2], F32, tag=t2, bufs=6)
                nc.vector.bn_aggr(st2[:], st6[:])
                sd = sb.tile([128, 1], F32, tag=tsd, bufs=6)
                nc.scalar.activation(sd[:], st2[:, 1:2], AF.Sqrt, bias=eps_t[:])
                rs = sb.tile([128, 1], F32, tag=trs, bufs=6)
                nc.vector.reciprocal(rs[:], sd[:])
                stats.append((st2, rs))
            for m in range(NMOD):
                st2, rs = stats[m]
                nc.vector.tensor_scalar(out=out[:, m, :], in0=src_tiles[m][:],
                                        scalar1=st2[:, 0:1], scalar2=rs[:],
                                        op0=AL.subtract, op1=AL.mult)
            return out

        def emit_A(rt):
            """Aux loads + feat load/cast/transpose + projection + relu."""
            r0 = rt * 128
            lg = sb.tile([128, NMOD, 7], F32, tag="lg", bufs=2)
            nc.gpsimd.dma_start(
                lg[:], logits_d.ap()[:, r0:r0 + 128, :].rearrange("m r c -> r m c"))
            nc.vector.tensor_copy(out=lgpad[rt % 2][:, 0:35],
                                  in_=lg[:].rearrange("p m c -> p (m c)"))
            lt = sb.tile([128, 128], BF16, tag="lt", bufs=8)
            nc.sync.dma_start(lt[:], lgpad[rt % 2][:], transpose=True)
            sc = sb.tile([128, NMOD, 5], F32, tag="sc", bufs=2)
            nc.gpsimd.dma_start(
                sc[:], scores5_d.ap()[:, r0:r0 + 128, :].rearrange("m r c -> r m c"))
            nc.vector.tensor_copy(out=scpad[rt % 2][:, 0:25],
                                  in_=sc[:].rearrange("p m c -> p (m c)"))
            st = sb.tile([128, 128], BF16, tag="st", bufs=8)
            nc.sync.dma_start(st[:], scpad[rt % 2][:], transpose=True)

            h_sb, fts = [], []
            for m in range(NMOD):
                nk = NK[m]
                fz = sb.tile([128, IN_DIMS[0]], BF16, tag="fz", bufs=2)
                nc.gpsimd.dma_start(
                    fz[:, :IN_DIMS[m]], feat_d[m].ap()[r0:r0 + 128, :])
                fT = sb.tile([128, NK[0], 128], BF16, tag="fT", bufs=2)
                nc.sync.dma_start(fT[:, :nk, :], fz[:, :IN_DIMS[m]],
                                  transpose=True)
                fts.append(fT)
            for m in range(NMOD):
                nk = NK[m]
                h_ps = ps.tile([128, H], F32, tag="psA", bufs=2)
                if flags["bp"]:
                    nc.tensor.matmul(h_ps[:], lhsT=ones1[:],
                                     rhs=bp_bf[:, m * H:(m + 1) * H],
                                     start=True, stop=False)
                for k in range(nk):
                    nc.tensor.matmul(h_ps[:], lhsT=fts[m][:, k, :],
                                     rhs=wp_bf[m][:, k, :],
                                     start=(k == 0 and not flags["bp"]),
                                     stop=(k == nk - 1))
                hs = sb.tile([128, H], BF16, tag="h_sb", bufs=6)
                nc.scalar.activation(hs[:], h_ps[:], AF.Relu)
                h_sb.append(hs)
            return dict(r0=r0, h_sb=h_sb, lt=lt, st=st)

        def emit_LN1(state):
            hln = layer_norm(state["h_sb"], "hln", "st6", "st2", "sd", "rs")
            hT = sb.tile([128, NMOD * 4, 128], BF16, tag="hT", bufs=2)
            nc.sync.dma_start(hT[:], hln[:].rearrange("p m h -> p (m h)"),
                              transpose=True)
            state["hln"] = hln
            state["hT"] = hT
            return state

        def emit_C(state):
            hT, hln = state["hT"], state["hln"]
            z = sb.tile([128, NMOD, 256], BF16, tag="z", bufs=2)
            for m in range(NMOD):
                a1_ps = ps.tile([128, 256], F32, tag="psB", bufs=2)
                if flags["ab1"]:
                    nc.tensor.matmul(a1_ps[:], lhsT=ones1[:],
                                     rhs=ab1_bf[:, m * 256:(m + 1) * 256],
                                     start=True, stop=False)
                for k in range(4):
                    nc.tensor.matmul(a1_ps[:], lhsT=hT[:, m * 4 + k, :],
                                     rhs=aw1_bf[:, m, k, :],
                                     start=(k == 0 and not flags["ab1"]),
                                     stop=(k == 3))
                nc.scalar.activation(z[:, m, :], a1_ps[:], AF.Relu)
            zT = sb.tile([128, NMOD * 2, 128], BF16, tag="zT", bufs=2)
            nc.sync.dma_start(zT[:], z[:].rearrange("p m h -> p (m h)"),
                              transpose=True)
            us = []
            for m in range(NMOD):
                a2_ps = ps.tile([128, H], F32, tag="psC", bufs=2)
                if flags["ab2"]:
                    nc.tensor.matmul(a2_ps[:], lhsT=ones1[:],
                                     rhs=ab2_bf[:, m * H:(m + 1) * H],
                                     start=True, stop=False)
                for k in range(2):
                    nc.tensor.matmul(a2_ps[:], lhsT=zT[:, m * 2 + k, :],
                                     rhs=aw2_bf[:, m, k, :],
                                     start=(k == 0 and not flags["ab2"]),
                                     stop=(k == 1))
                u = sb.tile([128, H], BF16, tag="u", bufs=6)
                nc.vector.tensor_tensor(out=u[:], in0=a2_ps[:],
                                        in1=hln[:, m, :], op=AL.add)
                us.append(u)
            state["us"] = us
            return state

        def emit_LN2(state):
            xm = layer_norm(state["us"], "xm", "st6b", "st2b", "sdb", "rsb")
            xT = sb.tile([128, NMOD * 4, 128], BF16, tag="xT", bufs=2)
            nc.sync.dma_start(xT[:], xm[:].rearrange("p m h -> p (m h)"),
                              transpose=True)
            state["xT"] = xT
            return state

        def emit_E(state):
            xT = state["xT"]
            xss = sb.tile([128, NMOD, 18], F32, tag="xss", bufs=3)
            xvt = sb.tile([128, H, NMOD], BF16, tag="xvt", bufs=2)
            for m in range(NMOD):
                xv_ps = ps.tile([128, H], F32, tag="psC", bufs=2)
                for k in range(4):
                    nc.tensor.matmul(xv_ps[:], lhsT=xT[:, m * 4 + k, :],
                                     rhs=gv_bf[:, k, :],
                                     start=(k == 0), stop=(k == 3))
                xs_ps = ps.tile([128, 18], F32, tag="psB", bufs=2)
                for k in range(4):
                    nc.tensor.matmul(xs_ps[:], lhsT=xT[:, m * 4 + k, :],
                                     rhs=gs_bf[:, k, :],
                                     start=(k == 0), stop=(k == 3))
                nc.scalar.activation(xvt[:, :, m], xv_ps[:], AF.Copy)
                nc.vector.tensor_copy(out=xss[:, m, :], in_=xs_ps[:])
            state.update(xss=xss, xvt=xvt)
            return state

        def emit_attn(state):
            """GAT attention softmaxes + pooled-attention weights."""
            xss = state["xss"]

            e = sb.tile([128, 5, 5], F32, tag="e", bufs=1)
            s2cat = xss[:, :, 17]
            for i in range(NMOD):
                nc.vector.tensor_scalar(out=e[:, i, :], in0=s2cat,
                                        scalar1=xss[:, i, 16:17], scalar2=None,
                                        op0=AL.add)
            el = sb.tile([128, 25], F32, tag="el", bufs=1)
            nc.vector.scalar_tensor_tensor(
                out=el[:], in0=e[:].rearrange("p a b -> p (a b)"), scalar=ALPHA,
                in1=e[:].rearrange("p a b -> p (a b)"), op0=AL.mult, op1=AL.max)
            ex = sb.tile([128, 5, 5], F32, tag="ex", bufs=1)
            nc.scalar.activation(ex[:].rearrange("p a b -> p (a b)"), el[:], AF.Exp)
            den = sb.tile([128, 5], F32, tag="den", bufs=1)
            nc.vector.tensor_reduce(out=den[:], in_=ex[:],
                                    axis=mybir.AxisListType.X, op=AL.add)
            rden = sb.tile([128, 5], F32, tag="rden", bufs=1)
            nc.vector.reciprocal(rden[:], den[:])
            attn = sb.tile([128, 5, 5], F32, tag="attn", bufs=1)
            nc.vector.tensor_tensor(
                out=attn[:], in0=ex[:],
                in1=rden[:, :, None].broadcast_to([128, 5, 5]), op=AL.mult)

            tmp400 = sb.tile([128, 16, 5, 5], F32, tag="tmp400", bufs=1)
            nc.vector.tensor_tensor(
                out=tmp400[:],
                in0=xss[:, :, 0:16].rearrange("p j q -> p q j")[:, :, None, :]
                    .broadcast_to([128, 16, 5, 5]),
                in1=attn[:][:, None, :, :].broadcast_to([128, 16, 5, 5]),
                op=AL.mult)
            S = sb.tile([128, 16, 5], F32, tag="S", bufs=2)
            nc.vector.tensor_reduce(out=S[:], in_=tmp400[:],
                                    axis=mybir.AxisListType.X, op=AL.add)
            if flags["ck"]:
                nc.vector.tensor_tensor(
                    out=S[:], in0=S[:],
                    in1=ck_t[:][:, :, None].broadcast_to([128, 16, 5]), op=AL.add)
            ES = sb.tile([128, 16, 5], F32, tag="ES", bufs=2)
            nc.scalar.activation(ES[:].rearrange("p a b -> p (a b)"),
                                 S[:].rearrange("p a b -> p (a b)"), AF.Exp)
            den16 = sb.tile([128, 16], F32, tag="den16", bufs=2)
            nc.vector.tensor_reduce(out=den16[:], in_=ES[:],
                                    axis=mybir.AxisListType.X, op=AL.add)
            rden16 = sb.tile([128, 16], F32, tag="rden16", bufs=2)
            nc.vector.reciprocal(rden16[:], den16[:])
            P = sb.tile([128, 16, 5], BF16, tag="P", bufs=2)
            nc.vector.tensor_tensor(
                out=P[:], in0=ES[:],
                in1=rden16[:, :, None].broadcast_to([128, 16, 5]), op=AL.mult)

            tmp2 = sb.tile([128, 16, 5, 5], BF16, tag="tmp2", bufs=1)
            nc.vector.tensor_tensor(
                out=tmp2[:],
                in0=P[:][:, :, None, :].broadcast_to([128, 16, 5, 5]),
                in1=attn[:].rearrange("p n j -> p j n")[:, None, :, :]
                    .broadcast_to([128, 16, 5, 5]),
                op=AL.mult)
            W = sb.tile([128, 16, 5], BF16, tag="W", bufs=3)
            with nc.allow_low_precision("5-term pooled-attn sums"):
                nc.vector.tensor_reduce(out=W[:], in_=tmp2[:],
                                        axis=mybir.AxisListType.X, op=AL.add)

            state["W"] = W
            return state

        def emit_pool(state):
            """Pooled values + out-proj + heads + store."""
            xvt, W = state["xvt"], state["W"]
            lt, st, r0 = state["lt"], state["st"], state["r0"]
            o_pair = sb.tile([128, 2, H], BF16, tag="o_pair", bufs=2)
            for q in range(2):
                tmp_o = sb.tile([128, NH, HD, 5], BF16, tag="tmp_o", bufs=2)
                nc.vector.tensor_tensor(
                    out=tmp_o[:],
                    in0=xvt[:].rearrange("p (h d) j -> p h d j", h=NH),
                    in1=W[:, q * 8:(q + 1) * 8, None, :]
                        .broadcast_to([128, NH, HD, 5]),
                    op=AL.mult)
                with nc.allow_low_precision("5-term pooled-attn sums"):
                    nc.vector.tensor_reduce(
                        out=o_pair[:, q, :].rearrange("p (h d) -> p h d", h=NH),
                        in_=tmp_o[:], axis=mybir.AxisListType.X, op=AL.add)
            oT = sb.tile([128, 8, 128], BF16, tag="oT", bufs=1)
            nc.sync.dma_start(oT[:], o_pair[:].rearrange("p a b -> p (a b)"),
                              transpose=True)
            rep_pair = sb.tile([128, 2, H], BF16, tag="rep_pair", bufs=2)
            rnorm = []
            for q in range(2):
                repr_ps = ps.tile([128, H], F32, tag="psD", bufs=2)
                if flags["rc"]:
                    nc.tensor.matmul(repr_ps[:], lhsT=ones1[:],
                                     rhs=rc_bf[:, q * H:(q + 1) * H],
                                     start=True, stop=False)
                for k in range(4):
                    nc.tensor.matmul(repr_ps[:], lhsT=oT[:, q * 4 + k, :],
                                     rhs=wo_bf[:, k, :],
                                     start=(k == 0 and not flags["rc"]),
                                     stop=False)
                if q == 0:
                    nc.tensor.matmul(repr_ps[:], lhsT=lt[0:35, :],
                                     rhs=elp5_bf[:], start=False, stop=True)
                else:
                    nc.tensor.matmul(repr_ps[:], lhsT=st[0:25, :],
                                     rhs=plp5_bf[:], start=False, stop=True)
                nc.scalar.activation(rep_pair[:, q, :], repr_ps[:], AF.Copy)
                sq = sb.tile([128, H], BF16, tag="tmp_o", bufs=2)
                n2 = sb.tile([128, 1], F32, tag=f"n2{q}", bufs=2)
                nc.vector.scalar_tensor_tensor(
                    out=sq[:], in0=rep_pair[:, q, :], scalar=1.0, in1=repr_ps[:],
                    op0=AL.mult, op1=AL.mult, accum_out=n2[:])
                nrm = sb.tile([128, 1], F32, tag=f"nrm{q}", bufs=2)
                nc.scalar.activation(nrm[:], n2[:], AF.Sqrt)
                nc.vector.tensor_scalar_max(nrm[:], nrm[:], 1e-8)
                rn = sb.tile([128, 1], F32, tag=f"rn{q}", bufs=2)
                nc.vector.reciprocal(rn[:], nrm[:])
                rnorm.append(rn)

            pred_ps = ps.tile([128, 24], F32, tag="psB", bufs=2)
            if flags["pcb"]:
                nc.tensor.matmul(pred_ps[:], lhsT=ones1[:], rhs=pcb_bf[:],
                                 start=True, stop=False)
            rT = sb.tile([128, 8, 128], BF16, tag="rT", bufs=1)
            nc.sync.dma_start(rT[:], rep_pair[:].rearrange("p a b -> p (a b)"),
                              transpose=True)
            for q in range(2):
                cols = slice(0, 14) if q == 0 else slice(14, 24)
                for k in range(4):
                    nc.tensor.matmul(pred_ps[:, cols], lhsT=rT[:, q * 4 + k, :],
                                     rhs=pc_bf[:, k, cols],
                                     start=(k == 0 and not flags["pcb"]),
                                     stop=(k == 3))
            pred = sb.tile([128, 24], F32, tag="pred", bufs=2)
            nc.vector.tensor_copy(out=pred[:], in_=pred_ps[:])

            outt = sb.tile([128, 12], F32, tag="outt", bufs=2)
            nc.vector.scalar_tensor_tensor(
                out=outt[:, 0:7], in0=pred[:, 7:14], scalar=rnorm[0][:],
                in1=pred[:, 0:7], op0=AL.mult, op1=AL.add)
            sigc = sb.tile([128, 5], F32, tag="sigc", bufs=2)
            nc.scalar.activation(sigc[:], pred[:, 19:24], AF.Sigmoid,
                                 scale=rnorm[1][:])
            sigp = sb.tile([128, 5], F32, tag="sigp", bufs=2)
            nc.scalar.activation(sigp[:], pred[:, 14:19], AF.Sigmoid)
            sum5 = sb.tile([128, 5], F32, tag="sum5", bufs=2)
            nc.vector.tensor_tensor(out=sum5[:], in0=sigc[:], in1=sigp[:],
                                    op=AL.add)
            nc.vector.tensor_scalar_mul(outt[:, 7:12], sum5[:], 0.5)
            nc.sync.dma_start(out_d.ap()[r0:r0 + 128, :], outt[:])

        stages = [emit_A, emit_LN1, emit_C, emit_LN2, emit_E, emit_attn,
                  emit_pool]
        nstg = len(stages)
        tiles_seq = [t for _ in range(repeat) for t in range(NT)]
        states = {}
        for tick in range(len(tiles_seq) + nstg - 1):
            for s_idx in reversed(range(nstg)):
                i = tick - s_idx
                if 0 <= i < len(tiles_seq):
                    if s_idx == 0:
                        states[i] = emit_A(tiles_seq[i])
                    else:
                        states[i] = stages[s_idx](states[i])
            if tick - nstg + 1 >= 0:
                states.pop(tick - nstg + 1, None)

    return nc


_CACHE = {}


def _host_prep(inputs):
    f32 = np.float32
    gat_W = inputs["gat_W"].astype(f32)
    gat_a = inputs["gat_a"].astype(f32)
    mha_in_w = inputs["mha_in_w"].astype(f32)
    mha_in_b = inputs["mha_in_b"].astype(f32)
    Wq, Wk, Wv = np.split(mha_in_w, 3, axis=1)
    bq, bk, bv = np.split(mha_in_b, 3)

    def score_mat(query):
        qv = (query.astype(f32) @ Wq + bq).reshape(NH, HD)
        A = np.stack([Wk[:, h * HD:(h + 1) * HD] @ qv[h] for h in range(NH)], 1)
        cK = np.array([bk[h * HD:(h + 1) * HD] @ qv[h] for h in range(NH)], f32)
        return A / np.sqrt(HD), cK / np.sqrt(HD)

    A_emo, ck_emo = score_mat(inputs["emo_query"])
    A_pkl, ck_pkl = score_mat(inputs["pkl_query"])
    gs = gat_W @ np.concatenate(
        [A_emo, A_pkl, gat_a[:H, None], gat_a[H:, None]], 1)
    gv = gat_W @ Wv
    ck = np.concatenate([ck_emo, ck_pkl]).astype(f32)

    ln1_g = inputs["ln1_g"].astype(f32)
    ln1_b = inputs["ln1_b"].astype(f32)
    ln2_g = inputs["ln2_g"].astype(f32)
    ln2_b = inputs["ln2_b"].astype(f32)
    ln1_trivial = np.allclose(ln1_g, 1.0) and np.allclose(ln1_b, 0.0)
    ln2_trivial = np.allclose(ln2_g, 1.0) and np.allclose(ln2_b, 0.0)
    if not (ln1_trivial and ln2_trivial):
        raise NotImplementedError("non-trivial LN affine not supported")

    aw1 = np.stack([np.diag(ln1_g[m]) @ inputs["aW1"][m].astype(f32)
                    for m in range(NMOD)])
    ab1e = inputs["ab1"].astype(f32) + np.einsum(
        "mk,mkn->mn", ln1_b, inputs["aW1"].astype(f32))
    aw2 = inputs["aW2"].astype(f32)
    ab2e = inputs["ab2"].astype(f32)

    mha_out_w = inputs["mha_out_w"].astype(f32)
    mha_out_b = inputs["mha_out_b"].astype(f32)
    rc = np.stack([
        mha_out_b + bv @ mha_out_w + inputs["elp_b"].astype(f32),
        mha_out_b + bv @ mha_out_w + inputs["plp_b"].astype(f32)])

    def norm_rows(g):
        g = g.astype(f32)
        n = np.maximum(np.linalg.norm(g, axis=-1, keepdims=True), 1e-8)
        return g / n

    gn_emo = norm_rows(inputs["guide_emo"])
    gn_pkl = norm_rows(inputs["guide_pkl"])
    pc = np.concatenate([
        inputs["emo_head_w"].astype(f32) * 0.5, gn_emo.T * 0.5,
        inputs["pkl_head_w"].astype(f32), gn_pkl.T], 1)
    pcb = np.concatenate([
        inputs["emo_head_b"].astype(f32) * 0.5, np.zeros(7, f32),
        inputs["pkl_head_b"].astype(f32), np.zeros(5, f32)])

    elp5 = np.tile(inputs["elp_w"].astype(f32) / NMOD, (NMOD, 1))
    plp5 = np.tile(inputs["plp_w"].astype(f32) / NMOD, (NMOD, 1))

    host = dict(
        gv=np.ascontiguousarray(gv, f32), gs=np.ascontiguousarray(gs, f32),
        wo=np.ascontiguousarray(mha_out_w, f32),
        pc=np.ascontiguousarray(pc, f32),
        elp5=np.ascontiguousarray(elp5, f32),
        plp5=np.ascontiguousarray(plp5, f32),
        aw1=np.ascontiguousarray(aw1, f32), aw2=np.ascontiguousarray(aw2, f32),
        bp=np.ascontiguousarray(inputs["bp"], f32),
        ab1e=np.ascontiguousarray(ab1e, f32),
        ab2e=np.ascontiguousarray(ab2e, f32),
        rc=np.ascontiguousarray(rc, f32),
        pcb=np.ascontiguousarray(pcb[None, :], f32),
        ck=np.ascontiguousarray(ck[None, :], f32),
    )
    flags = dict(
        bp=not np.allclose(host["bp"], 0.0),
        ab1=not np.allclose(host["ab1e"], 0.0),
        ab2=not np.allclose(host["ab2e"], 0.0),
        rc=not np.allclose(host["rc"], 0.0),
        pcb=not np.allclose(host["pcb"], 0.0),
        ck=not np.allclose(host["ck"], 0.0),
    )
    return host, flags


def _run(inputs, **spmd_kwargs):
    from concourse.bass_utils import run_bass_kernel_spmd

    host, flags = _host_prep(inputs)
    key = tuple(sorted(flags.items()))
    if key not in _CACHE:
        _CACHE[key] = _build_nc(flags)
    nc = _CACHE[key]

    in_maps = []
    for c in range(NCORES):
        r = slice(c * B_CORE, (c + 1) * B_CORE)
        im = {f"feat_{m}": np.ascontiguousarray(
                  inputs[f"feat_{m}"][r], np.float32) for m in MODS}
        for m in MODS:
            im[f"wp_{m}"] = np.ascontiguousarray(inputs[f"Wp_{m}"], np.float32)
        im["logits"] = np.ascontiguousarray(
            inputs["emo_logits_all"][:, r, :], np.float32)
        im["scores5"] = np.ascontiguousarray(
            inputs["per_scores_all"][:, r, :], np.float32)
        im.update(host)
        in_maps.append(im)

    res = run_bass_kernel_spmd(nc, in_maps, list(range(NCORES)), **spmd_kwargs)
    out = np.concatenate([res.results[c]["out"] for c in range(NCORES)], 0)
    return out, res


def kernel(**inputs):
    return _run(inputs)[0]

